# revision 5
# baseline (speedup 1.0000x reference)
"""GATv2 GNN (3 layers + per-graph max readout + MLP classifier) on 8 trn2 NeuronCores.

Sharding: nodes partitioned contiguously across 8 cores (6250 each). Edges are
bucketed by destination-node partition and sorted by dst; per dst-tile (128
nodes) the incoming edges form a contiguous run, padded to 128-edge chunks on a
schedule shared by all cores (SPMD: one program).

Per layer, per core:
  node phase : h^T tiles via PE transpose -> hs/hd = h @ Wsrc/Wdst + b (PE, bf16)
  AllGather  : hs shards -> full hs table [50000,256] bf16 in local DRAM
  edge phase : per 128-edge chunk, indirect-DMA gather hs[src] and hd[dst_local];
               ef=lrelu(hs+hd) (DVE+ACT Prelu); e = <ef,attn> (DVE mult+reduce);
               ex=exp(e) (ACT); rhs=[ex*hs | ex]; segment-sum via PE matmul with
               a host-built one-hot edge->dst matrix PT into PSUM [128 dst, 264]
  update     : rst = num/denom ; h = relu(rst + h)
Readout: per-graph max over local nodes (PE transpose + DVE reduce, graphs are
contiguous since graph_ids is sorted), AllReduce(max), replicated classifier.
"""
import hashlib
import numpy as np
import ml_dtypes

import concourse.bass as bass
import concourse.bacc as bacc
import concourse.tile as tile
import concourse.mybir as mybir
from concourse.masks import make_identity
from concourse.bass_utils import run_bass_kernel_spmd

F32 = mybir.dt.float32
BF16 = mybir.dt.bfloat16
I32 = mybir.dt.int32
BF = ml_dtypes.bfloat16

NCORES = 8
N = 50000
E = 800000
G = 64
IN_DIM = 128
HID = 256
HEADS = 8
DH = 32
OUT = 10
NEG = 0.2
NLOC = N // NCORES          # 6250
TILE = 128
NT = (NLOC + TILE - 1) // TILE   # 49
NLOC_PAD = NT * TILE             # 6272
GRP = 8                          # chunks per group (DVE/ACT amortization)
EPS = 1e-30


# ----------------------------------------------------------------------------- host prep

def _preprocess(x, src, dst, graph_ids):
    """Build per-core shard data + the shared chunk schedule."""
    src = np.asarray(src, np.int64)
    dst = np.asarray(dst, np.int64)
    gid = np.asarray(graph_ids, np.int64)

    core_of = dst // NLOC
    # per-core sorted edge lists
    per_core = []
    R = np.zeros((NCORES, NT), np.int64)
    for c in range(NCORES):
        m = core_of == c
        s_c = src[m]
        d_c = dst[m] - c * NLOC
        o = np.argsort(d_c, kind="stable")
        s_c, d_c = s_c[o], d_c[o]
        per_core.append((s_c, d_c))
        R[c] = np.bincount(d_c // TILE, minlength=NT)

    C = np.maximum.reduce([np.ceil(R[c] / TILE).astype(np.int64) for c in range(NCORES)])
    assert (C > 0).all(), "some dst tile has zero edges on all cores"
    CC = int(C.sum())
    NGRP = (CC + GRP - 1) // GRP
    CCg = NGRP * GRP
    C = C.copy()
    C[NT - 1] += CCg - CC           # pad chunks appended to the last tile
    CC = CCg
    chunk_tile = np.repeat(np.arange(NT), C)        # [CC]
    tile_chunk_start = np.concatenate([[0], np.cumsum(C)])[:-1]

    srcidx = np.zeros((NCORES, 128, CC), np.int32)
    dstidx = np.zeros((NCORES, 128, CC), np.int32)
    PT = np.zeros((NCORES, CC * 128, 128), BF)
    for c in range(NCORES):
        s_c, d_c = per_core[c]
        tiles = d_c // TILE
        cnt = np.bincount(tiles, minlength=NT)
        start_e = np.concatenate([[0], np.cumsum(cnt)])[:-1]
        # slot index of each edge: tile_chunk_start[t]*128 + rank within tile
        rank = np.arange(len(d_c)) - start_e[tiles]
        slot = tile_chunk_start[tiles] * 128 + rank
        si = np.zeros(CC * 128, np.int32)
        di = np.zeros(CC * 128, np.int32)
        si[slot] = s_c
        di[slot] = d_c
        srcidx[c] = si.reshape(CC, 128).T
        dstidx[c] = di.reshape(CC, 128).T
        col = d_c - tiles * TILE
        PT[c][slot, col] = 1.0
    # group layout for PT: [NGRP, 128, GRP*128] with chunk j at cols j*128
    PTg = PT.reshape(NCORES, NGRP, GRP, 128, 128).transpose(0, 1, 3, 2, 4).reshape(
        NCORES, NGRP, 128, GRP * 128)

    # per-core transposed x, zero-padded to NLOC_PAD
    x = np.asarray(x, np.float32)
    xT = np.zeros((NCORES, IN_DIM, NLOC_PAD), BF)
    for c in range(NCORES):
        xT[c, :, :NLOC] = x[c * NLOC:(c + 1) * NLOC].T.astype(BF)

    # graph segments per core per tile: (tile, gid, lo, hi) with lo/hi in [0,128]
    segs = []
    for c in range(NCORES):
        g_c = gid[c * NLOC:(c + 1) * NLOC]
        lst = []
        for t in range(NT):
            lo_n = t * TILE
            hi_n = min((t + 1) * TILE, NLOC)
            if lo_n >= hi_n:
                continue
            gt = g_c[lo_n:hi_n]
            bounds = np.flatnonzero(np.diff(gt)) + 1
            starts = np.concatenate([[0], bounds])
            ends = np.concatenate([bounds, [hi_n - lo_n]])
            for s, e2 in zip(starts, ends):
                lst.append((t, int(gt[s]), int(s), int(e2)))
        segs.append(lst)

    meta = dict(CC=CC, NGRP=NGRP, C=C, chunk_tile=chunk_tile, segs=segs)
    shards = dict(srcidx=srcidx, dstidx=dstidx, PT=PTg, xT=xT)
    return meta, shards


def _pack_weights(Win, b_in, Wsrc, bsrc, Wdst, bdst, attn, Wc1, bc1, Wc2, bc2, Wc3, bc3):
    w = {}
    w["Win_bf"] = np.asarray(Win, np.float32).astype(BF)                       # [128,256]
    Wsrc = np.asarray(Wsrc, np.float32)
    Wdst = np.asarray(Wdst, np.float32)
    # [l, p, kb*256+j] = W[l, kb*128+p, j]
    w["Wsrc_pk"] = Wsrc.reshape(3, 2, 128, 256).transpose(0, 2, 1, 3).reshape(3, 128, 512).astype(BF)
    w["Wdst_pk"] = Wdst.reshape(3, 2, 128, 256).transpose(0, 2, 1, 3).reshape(3, 128, 512).astype(BF)
    attn = np.asarray(attn, np.float32).reshape(3, HID)
    w["attn_bc"] = np.broadcast_to(attn[:, None, :], (3, 128, HID)).astype(BF).copy()
    w["b_in_bc"] = np.broadcast_to(np.asarray(b_in, np.float32)[None, :], (128, HID)).copy()
    w["bsrc_bc"] = np.broadcast_to(np.asarray(bsrc, np.float32)[:, None, :], (3, 128, HID)).copy()
    w["bdst_bc"] = np.broadcast_to(np.asarray(bdst, np.float32)[:, None, :], (3, 128, HID)).copy()
    Wc1 = np.asarray(Wc1, np.float32)
    # [p, (kb*2+mb)*128+j] = Wc1[kb*128+p, mb*128+j]
    w["Wc1_pk"] = Wc1.reshape(2, 128, 2, 128).transpose(1, 0, 2, 3).reshape(128, 512).astype(BF)
    Wc2 = np.asarray(Wc2, np.float32)
    w["Wc2_pk"] = Wc2.reshape(2, 128, 128).transpose(1, 0, 2).reshape(128, 256).astype(BF)
    w["Wc3_bf"] = np.asarray(Wc3, np.float32).astype(BF)                        # [128,10]
    w["bc1_col"] = np.asarray(bc1, np.float32).reshape(2, 128).T.copy()         # [128,2]
    w["bc2_col"] = np.asarray(bc2, np.float32).reshape(128, 1).copy()           # [128,1]
    w["bc3_bc"] = np.broadcast_to(np.asarray(bc3, np.float32)[None, :], (G, OUT)).copy()
    return w


# ----------------------------------------------------------------------------- builder

def build_program_v2(meta, debug_taps=False):
    """SPMD-safe build. Readout handled with a per-core DATA-driven scheme:

    The per-graph max is computed as gmax[f, g] = max over nodes n of
      h[n, f] * M[n, g]  ... (mult by 0/1 mask, then max-reduce)
    with M a per-core one-hot [128, 64] f32 mask per tile (node->graph),
    shipped as input data. Since h >= 0 (post-relu) and every graph column
    with no nodes stays 0 (matching the reference's empty->0), a masked
    multiply + max-reduce gives the exact per-graph max.

    Implementation per tile t, per feature block b (feat-major after PE
    transpose tp [128 f, 128 n]):
      for gcol in active 64-col window: DVE op out[128,64]? needs per-(f,g)
      max over n: tmp[f, n, g]? too big.

    Cheaper: per GRAPH g, the node set is a contiguous [lo,hi) range --
    identical op shape if we always reduce the FULL tile [0,128) after adding
    a mask: tpm = tp * mask_g (mask per node 0/1 broadcast over f) then
    reduce-max. But mask differs per core only in DATA (input tensor), the
    instruction stream is fixed: per tile we emit KSEG reduce+max-merge pairs
    where KSEG = max segments/tile over cores (schedule shared), the n-range
    is always the full tile, the mask column used comes from an input tensor
    (per-core data), and the merge target column is selected by ... APs again.

    => Final simplification that is fully SPMD-uniform: merge-target indexing
    via a 64-wide one-hot matmul is overkill; instead compute per-tile
    "masked" gmax for ALL 64 graphs at once with 64 DVE ops... too many.

    PRACTICAL resolution: graph_ids is sorted GLOBALLY, so on every core the
    local graphs are a contiguous range [g_lo_c, g_hi_c] and tile boundaries
    differ. BUT the *schedule* (number of segments per tile and their (lo,hi,
    col)) is per-core data baked into APs. run_bass_kernel_spmd compiles ONE
    program for all cores, so APs cannot differ. We sidestep the whole issue:
    the readout is mathematically max(h[n,f]*m) and we evaluate it as a
    *sequence of 64 masked reductions* only over tiles that contain the
    graph's nodes on ANY core... still per-core ranges.

    TRUE final scheme (uniform, data-driven, cheap): host ships per-core
    GMASK [NT, 128, 64] bf16 (node-in-tile x graph one-hot). Per tile t and
    feature block b:
        tp   = transpose(h_b)            [128 f, 128 n]  (PSUM, f32)
        tpsb = copy(tp) -> bf16 SBUF     [128 f, 128 n]
        pm   = matmul(lhsT=tpsb? ...)    -- matmul gives SUM not MAX. no.
    so masked-max must be elementwise: for g in local window (<= ~2 graphs
    per tile usually, but padded to max over cores ~4): mask column as
    per-partition scalar? mask is per-NODE (free dim of tp). DVE
    tensor_tensor(mult) of tp [128,128] with mask_bcast [1? ...] mask is
    [n] -> broadcast over f (partitions): in1 = GMASK[t, :, gslot] as SBUF
    [128? it's per free-dim element...
    in1 AP: [128 part step 0, 128 n step 1]? partition-step-0 is not allowed
    on DVE.

    => mask must be materialized per-partition: host ships GMASKB [NT, 128,
    KSEGMAX*128] bf16? 49*4*128*128*2B = 12.8MB -- fine! GMASKB[t, p, s*128+n]
    = 1 if (node n of tile t belongs to the graph of slot s on this core).
    Then per (t, b, s): DVE mult tp*GMASKB[t,:,s] -> masked [128,128] then
    reduce-max -> [128,1] then max-merge into gmax[:, ???]  -- merge COLUMN
    still per-core. Merge via one-hot matmul (sum == select since disjoint):
    collect per-slot maxima into SM [128 f, NSLOT] then gmax = SM @ SEL with
    SEL [NSLOT, 64] one-hot per-core data... SUM-select works only if each
    graph appears in exactly the slots that... a graph may span multiple
    slots (tiles) -> SUM of maxima != max of maxima. BUT h>=0 and maxima of
    SAME graph from different tiles: sum over slots >= true max. WRONG.

    OK -- use two-level: per-slot maxima SM [128, NSLOT_TOT] (uniform stream)
    then do the per-graph combine ON HOST? The output of the kernel would
    then be per-slot maxima + host max-combine + host classifier matmul...
    The classifier is 99.9% of nothing; but doing it on host is against the
    spirit (it's real FLOPs, tiny). Alternative: combine on device via
    LOG-SUM trick? max(a,b) via matmul impossible.

    Chosen: per-slot maxima -> all 8 cores AllGather their SM -> every core
    computes... combine still needs per-core-data-driven maxing.

    SIMPLEST CORRECT: make the readout slot schedule GLOBALLY UNIFORM by
    slotting per GRAPH x TILE-OF-GRAPH: globally, graph g spans a contiguous
    node range [lo_g, hi_g) -> global tiles T0_g..T1_g. On the core owning a
    tile, that (g, tile) pair is a slot; cores not owning it emit the same
    instruction on zero data. I.e. schedule slots = global list of
    (core_tile, graph) pairs from the GLOBAL graph/tile structure -- SAME for
    all cores; each core's GMASKB has 1s only where it owns the nodes.
    Per-slot op: masked mult+reduce over the core-local tile `t(slot)` =
    (global_tile % NT)?? global tile index maps to (core, local tile) -- only
    ONE core owns it; other cores multiply by zero mask -> their partial is 0
    <= true max (h>=0): the final AllReduce(max) over cores fixes everything!

    And the merge column for slot is graph g -- same for all cores. The
    number of (tile, graph) slots globally: <= NT*8 + 63 ~ 455; per slot ops:
    1 mult [128,128] + 1 reduce + 1 max-merge ~ 455*3 DVE ops ~ 40us. OK.
    Masks: GMASKB indexed by slot: [NSLOT, 128, 128] bf16 = 455*32KB = 15MB.
    Cores not owning a slot have all-zero mask; owning core has node one-hot
    (diagonal-ish). Actually mask multiplies tp [128 f, 128 n] elementwise
    with mask[n] broadcast over f: per-partition broadcast impossible ->
    ship mask already broadcast: GMASKB[slot, p, n] = mask[n] for all p.
    15MB input per core, read once: ~40us DMA. acceptable.
    """
    CC, NGRP = meta["CC"], meta["NGRP"]
    C, chunk_tile = meta["C"], meta["chunk_tile"]
    slots = meta["slots"]          # list of (local_tile, graph) global slot schedule
    NSLOT = len(slots)

    nc = bacc.Bacc("TRN2", target_bir_lowering=False, debug=False, num_devices=NCORES)

    # ---- inputs
    ap = {}
    def din(name, shape, dt):
        ap[name] = nc.dram_tensor(name, shape, dt, kind="ExternalInput").ap()
    din("xT", [IN_DIM, NLOC_PAD], BF16)
    din("srcidx", [128, CC], I32)
    din("dstidx", [128, CC], I32)
    din("PT", [NGRP, 128, GRP * 128], BF16)
    din("Win_bf", [128, 256], BF16)
    din("Wsrc_pk", [3, 128, 512], BF16)
    din("Wdst_pk", [3, 128, 512], BF16)
    din("attn_bc", [3, 128, 256], BF16)
    din("b_in_bc", [128, 256], F32)
    din("bsrc_bc", [3, 128, 256], F32)
    din("bdst_bc", [3, 128, 256], F32)
    din("gmaskb", [NSLOT, 128, 128], BF16)
    din("Wc1_pk", [128, 512], BF16)
    din("Wc2_pk", [128, 256], BF16)
    din("Wc3_bf", [128, OUT], BF16)
    din("bc1_col", [128, 2], F32)
    din("bc2_col", [128, 1], F32)
    din("bc3_bc", [G, OUT], F32)

    out_ap = nc.dram_tensor("out", [G, OUT], F32, kind="ExternalOutput").ap()
    taps = {}
    if debug_taps:
        for nm in ("h0", "h1", "h2", "h3"):
            taps[nm] = nc.dram_tensor(nm, [NLOC_PAD, HID], F32, kind="ExternalOutput").ap()
        taps["gmax"] = nc.dram_tensor("tap_gmax", [128, 128], F32, kind="ExternalOutput").ap()

    # ---- internal DRAM
    hs_bounce = [nc.dram_tensor(f"hs_bounce{l}", [NLOC, HID], BF16, kind="Internal").ap() for l in range(3)]
    hs_full = [nc.dram_tensor(f"hs_full{l}", [N, HID], BF16, kind="Internal", addr_space="Shared").ap() for l in range(3)]
    hd_dram = [nc.dram_tensor(f"hd{l}", [NLOC, HID], BF16, kind="Internal").ap() for l in range(3)]
    gm_in = nc.dram_tensor("gm_in", [128, 128], F32, kind="Internal").ap()
    gm_out = nc.dram_tensor("gm_out", [128, 128], F32, kind="Internal", addr_space="Shared").ap()

    with tile.TileContext(nc) as tc:
        with (
            tc.tile_pool(name="const", bufs=1) as cp,
            tc.tile_pool(name="hbuf", bufs=1) as hp,
            tc.tile_pool(name="node", bufs=3) as npl,
            tc.tile_pool(name="edge", bufs=3) as ep,
            tc.tile_pool(name="upd", bufs=2) as up,
            tc.tile_pool(name="ro", bufs=3) as rp,
            tc.tile_pool(name="psA", bufs=2, space="PSUM") as psA,   # transposes
            tc.tile_pool(name="psB", bufs=2, space="PSUM") as psB,   # node matmuls
            tc.tile_pool(name="psS", bufs=2, space="PSUM") as psS,   # segment accum
        ):
            # ------- constants to SBUF
            def load_const(name, shape, dt):
                t = cp.tile(shape, dt, name=f"c_{name}", tag=f"c_{name}")
                nc.sync.dma_start(t[:], ap[name][:])
                return t
            xT_sb = load_const("xT", [IN_DIM, NLOC_PAD], BF16)
            src_sb = load_const("srcidx", [128, CC], I32)
            dst_sb = load_const("dstidx", [128, CC], I32)
            Win_sb = load_const("Win_bf", [128, 256], BF16)
            Wsrc_sb = [None] * 3
            Wdst_sb = [None] * 3
            attn_sb = [None] * 3
            bsrc_sb = [None] * 3
            bdst_sb = [None] * 3
            for l in range(3):
                Wsrc_sb[l] = cp.tile([128, 512], BF16, tag=f"wsrc{l}", name=f"wsrc{l}")
                nc.sync.dma_start(Wsrc_sb[l][:], ap["Wsrc_pk"][l])
                Wdst_sb[l] = cp.tile([128, 512], BF16, tag=f"wdst{l}", name=f"wdst{l}")
                nc.sync.dma_start(Wdst_sb[l][:], ap["Wdst_pk"][l])
                attn_sb[l] = cp.tile([128, 256], BF16, tag=f"attn{l}", name=f"attn{l}")
                nc.sync.dma_start(attn_sb[l][:], ap["attn_bc"][l])
                bsrc_sb[l] = cp.tile([128, 256], F32, tag=f"bsrc{l}", name=f"bsrc{l}")
                nc.sync.dma_start(bsrc_sb[l][:], ap["bsrc_bc"][l])
                bdst_sb[l] = cp.tile([128, 256], F32, tag=f"bdst{l}", name=f"bdst{l}")
                nc.sync.dma_start(bdst_sb[l][:], ap["bdst_bc"][l])
            binc_sb = load_const("b_in_bc", [128, 256], F32)
            Wc1_sb = load_const("Wc1_pk", [128, 512], BF16)
            Wc2_sb = load_const("Wc2_pk", [128, 256], BF16)
            Wc3_sb = load_const("Wc3_bf", [128, OUT], BF16)
            bc1_sb = load_const("bc1_col", [128, 2], F32)
            bc2_sb = load_const("bc2_col", [128, 1], F32)
            bc3_sb = load_const("bc3_bc", [G, OUT], F32)
            ident = cp.tile([128, 128], F32)
            make_identity(nc, ident)
            alpha_sb = cp.tile([128, 1], F32)
            nc.vector.memset(alpha_sb[:], NEG)

            h_sb = hp.tile([128, NT * 256], F32)

            # ------- phase 0: input projection
            for t in range(NT):
                ps = psB.tile([128, 256], F32, tag="mmA")
                nc.tensor.matmul(out=ps[:], lhsT=xT_sb[:, t * 128:(t + 1) * 128],
                                 rhs=Win_sb[:], start=True, stop=True)
                nc.vector.tensor_tensor(out=h_sb[:, t * 256:(t + 1) * 256],
                                        in0=ps[:], in1=binc_sb[:], op=mybir.AluOpType.add)
            if debug_taps:
                for t in range(NT):
                    nc.sync.dma_start(taps["h0"][t * 128:(t + 1) * 128, :], h_sb[:, t * 256:(t + 1) * 256])

            # ------- layers
            for l in range(3):
                # node phase
                for t in range(NT):
                    hT = npl.tile([128, 256], BF16, tag="hT")
                    for kb in range(2):
                        tp = psA.tile([128, 128], F32, tag="tp")
                        nc.tensor.transpose(out=tp[:], in_=h_sb[:, t * 256 + kb * 128: t * 256 + (kb + 1) * 128],
                                            identity=ident[:])
                        nc.vector.tensor_copy(hT[:, kb * 128:(kb + 1) * 128], tp[:])
                    hs_ps = psB.tile([128, 256], F32, tag="mmA")
                    hd_ps = psB.tile([128, 256], F32, tag="mmB")
                    for kb in range(2):
                        nc.tensor.matmul(out=hs_ps[:], lhsT=hT[:, kb * 128:(kb + 1) * 128],
                                         rhs=Wsrc_sb[l][:, kb * 256:(kb + 1) * 256],
                                         start=(kb == 0), stop=(kb == 1))
                    for kb in range(2):
                        nc.tensor.matmul(out=hd_ps[:], lhsT=hT[:, kb * 128:(kb + 1) * 128],
                                         rhs=Wdst_sb[l][:, kb * 256:(kb + 1) * 256],
                                         start=(kb == 0), stop=(kb == 1))
                    hs_o = npl.tile([128, 256], BF16, tag="hs_o")
                    nc.vector.tensor_tensor(out=hs_o[:], in0=hs_ps[:], in1=bsrc_sb[l][:], op=mybir.AluOpType.add)
                    hd_o = npl.tile([128, 256], BF16, tag="hd_o")
                    nc.vector.tensor_tensor(out=hd_o[:], in0=hd_ps[:], in1=bdst_sb[l][:], op=mybir.AluOpType.add)
                    rows = min(TILE, NLOC - t * TILE)
                    nc.sync.dma_start(hs_bounce[l][t * TILE:t * TILE + rows, :], hs_o[:rows, :])
                    nc.sync.dma_start(hd_dram[l][t * TILE:t * TILE + rows, :], hd_o[:rows, :])

                nc.gpsimd.collective_compute(
                    "AllGather", mybir.AluOpType.bypass,
                    replica_groups=[list(range(NCORES))],
                    ins=[hs_bounce[l][:]], outs=[hs_full[l][:]],
                )

                # edge phase
                open_ps = {}
                done = np.zeros(NT, np.int64)
                for g in range(NGRP):
                    pt = ep.tile([128, GRP * 128], BF16, tag="pt")
                    nc.sync.dma_start(pt[:], ap["PT"][g])
                    hsg = ep.tile([128, GRP * 256], BF16, tag="hsg")
                    hdg = ep.tile([128, GRP * 256], BF16, tag="hdg")
                    for j in range(GRP):
                        ch = g * GRP + j
                        nc.gpsimd.indirect_dma_start(
                            out=hsg[:, j * 256:(j + 1) * 256], out_offset=None,
                            in_=hs_full[l][:],
                            in_offset=bass.IndirectOffsetOnAxis(ap=src_sb[:, ch:ch + 1], axis=0))
                        nc.gpsimd.indirect_dma_start(
                            out=hdg[:, j * 256:(j + 1) * 256], out_offset=None,
                            in_=hd_dram[l][:],
                            in_offset=bass.IndirectOffsetOnAxis(ap=dst_sb[:, ch:ch + 1], axis=0))
                    nc.vector.tensor_tensor(out=hdg[:], in0=hdg[:], in1=hsg[:], op=mybir.AluOpType.add)
                    nc.scalar.activation(out=hdg[:], in_=hdg[:],
                                         func=mybir.ActivationFunctionType.Prelu,
                                         alpha=alpha_sb[:, 0:1])
                    attn3 = attn_sb[l][:].rearrange("p (o c) -> p o c", o=1).to_broadcast([128, GRP, 256])
                    nc.vector.tensor_tensor(
                        out=hdg[:].rearrange("p (j c) -> p j c", c=256),
                        in0=hdg[:].rearrange("p (j c) -> p j c", c=256),
                        in1=attn3, op=mybir.AluOpType.mult)
                    e32 = ep.tile([128, GRP * 8], F32, tag="e32")
                    nc.vector.tensor_reduce(out=e32[:], in_=hdg[:].rearrange("p (a d) -> p a d", d=32),
                                            axis=mybir.AxisListType.X, op=mybir.AluOpType.add)
                    ex = ep.tile([128, GRP * 8], BF16, tag="ex")
                    nc.scalar.activation(out=ex[:], in_=e32[:], func=mybir.ActivationFunctionType.Exp)
                    rhs = ep.tile([128, GRP * 264], BF16, tag="rhs")
                    nc.vector.tensor_copy(
                        rhs[:].rearrange("p (j c) -> p j c", c=264)[:, :, 256:264],
                        ex[:].rearrange("p (j c) -> p j c", c=8))
                    nc.vector.tensor_tensor(
                        out=rhs[:].rearrange("p (j c) -> p j c", c=264)[:, :, 0:256]
                            .rearrange("p j (h d) -> p j h d", d=32),
                        in0=hsg[:].rearrange("p (j h d) -> p j h d", h=8, d=32),
                        in1=ex[:].rearrange("p (j h) -> p j h", h=8)
                            .rearrange("p j (h o) -> p j h o", o=1).to_broadcast([128, GRP, 8, 32]),
                        op=mybir.AluOpType.mult)
                    for j in range(GRP):
                        ch = g * GRP + j
                        t = int(chunk_tile[ch])
                        if t not in open_ps:
                            open_ps[t] = psS.tile([128, 264], F32, tag="seg", name=f"seg_l{l}_t{t}")
                        first = done[t] == 0
                        done[t] += 1
                        last = done[t] == C[t]
                        nc.tensor.matmul(out=open_ps[t][:],
                                         lhsT=pt[:, j * 128:(j + 1) * 128],
                                         rhs=rhs[:, j * 264:(j + 1) * 264],
                                         start=first, stop=last)
                        if last:
                            ps = open_ps.pop(t)
                            den = up.tile([128, 8], F32, tag="den")
                            nc.vector.tensor_scalar_add(out=den[:], in0=ps[:, 256:264], scalar1=EPS)
                            rec = up.tile([128, 8], F32, tag="rec")
                            nc.vector.reciprocal(rec[:], den[:])
                            updt = up.tile([128, 256], F32, tag="updt")
                            nc.vector.tensor_tensor(
                                out=updt[:].rearrange("p (h d) -> p h d", d=32),
                                in0=ps[:, 0:256].rearrange("p (h d) -> p h d", d=32),
                                in1=rec[:].rearrange("p (h o) -> p h o", o=1).to_broadcast([128, 8, 32]),
                                op=mybir.AluOpType.mult)
                            nc.vector.tensor_tensor(out=updt[:], in0=updt[:],
                                                    in1=h_sb[:, t * 256:(t + 1) * 256],
                                                    op=mybir.AluOpType.add)
                            nc.scalar.activation(out=h_sb[:, t * 256:(t + 1) * 256], in_=updt[:],
                                                 func=mybir.ActivationFunctionType.Relu)
                if debug_taps:
                    for t in range(NT):
                        nc.sync.dma_start(taps[f"h{l + 1}"][t * 128:(t + 1) * 128, :], h_sb[:, t * 256:(t + 1) * 256])

            # ------- readout: per-graph max (feat-major), slots are global schedule
            gmax = rp.tile([128, 128], F32, tag="gmax")
            nc.vector.memset(gmax[:], 0.0)
            # group slots by tile so we transpose each tile once per block
            from collections import defaultdict
            by_tile = defaultdict(list)
            for si, (t, g) in enumerate(slots):
                by_tile[t].append((si, g))
            for t in sorted(by_tile):
                for kb in range(2):
                    tp = psA.tile([128, 128], F32, tag="tp")
                    nc.tensor.transpose(out=tp[:], in_=h_sb[:, t * 256 + kb * 128: t * 256 + (kb + 1) * 128],
                                        identity=ident[:])
                    for (si, g) in by_tile[t]:
                        msk = rp.tile([128, 128], BF16, tag="msk")
                        nc.sync.dma_start(msk[:], ap["gmaskb"][si])
                        mskd = rp.tile([128, 128], F32, tag="mskd")
                        nc.vector.tensor_tensor(out=mskd[:], in0=tp[:], in1=msk[:], op=mybir.AluOpType.mult)
                        red = rp.tile([128, 1], F32, tag="red")
                        nc.vector.tensor_reduce(out=red[:], in_=mskd[:], axis=mybir.AxisListType.X,
                                                op=mybir.AluOpType.max)
                        col = kb * 64 + g
                        nc.vector.tensor_tensor(out=gmax[:, col:col + 1], in0=gmax[:, col:col + 1],
                                                in1=red[:], op=mybir.AluOpType.max)
            nc.sync.dma_start(gm_in[:], gmax[:])
            nc.gpsimd.collective_compute(
                "AllReduce", mybir.AluOpType.max,
                replica_groups=[list(range(NCORES))],
                ins=[gm_in[:]], outs=[gm_out[:]],
            )
            gT = rp.tile([128, 128], F32, tag="gT")
            nc.sync.dma_start(gT[:], gm_out[:])
            if debug_taps:
                nc.sync.dma_start(taps["gmax"][:], gT[:])
            gTb = rp.tile([128, 128], BF16, tag="gTb")
            nc.vector.tensor_copy(gTb[:], gT[:])

            # ------- classifier (replicated)
            z1 = rp.tile([128, 128], BF16, tag="z1")
            for mb in range(2):
                ps = psB.tile([128, 64], F32, tag="mmA")
                for kb in range(2):
                    nc.tensor.matmul(out=ps[:], lhsT=Wc1_sb[:, (kb * 2 + mb) * 128:(kb * 2 + mb + 1) * 128],
                                     rhs=gTb[:, kb * 64:(kb + 1) * 64], start=(kb == 0), stop=(kb == 1))
                nc.scalar.activation(out=z1[:, mb * 64:(mb + 1) * 64], in_=ps[:],
                                     func=mybir.ActivationFunctionType.Relu,
                                     bias=bc1_sb[:, mb:mb + 1], scale=1.0)
            ps2 = psB.tile([128, 64], F32, tag="mmB")
            for kb in range(2):
                nc.tensor.matmul(out=ps2[:], lhsT=Wc2_sb[:, kb * 128:(kb + 1) * 128],
                                 rhs=z1[:, kb * 64:(kb + 1) * 64], start=(kb == 0), stop=(kb == 1))
            z2 = rp.tile([128, 64], BF16, tag="z2")
            nc.scalar.activation(out=z2[:], in_=ps2[:], func=mybir.ActivationFunctionType.Relu,
                                 bias=bc2_sb[:, 0:1], scale=1.0)
            ps3 = psB.tile([64, OUT], F32, tag="mmA")
            nc.tensor.matmul(out=ps3[:], lhsT=z2[:, 0:64], rhs=Wc3_sb[:, 0:OUT], start=True, stop=True)
            out_sb = rp.tile([64, OUT], F32, tag="osb")
            nc.vector.tensor_tensor(out=out_sb[:], in0=ps3[:], in1=bc3_sb[:], op=mybir.AluOpType.add)
            nc.sync.dma_start(out_ap[:], out_sb[:])

    nc.compile()
    return nc


# ----------------------------------------------------------------------------- slots + masks

def _build_slots_and_masks(graph_ids):
    """Global (core, local_tile, graph) slot schedule + per-core masks.

    Slots enumerate, over the GLOBAL node order, each (owning-core tile,
    graph) incidence. The instruction schedule (which local tile to reduce,
    which gmax column to merge) is identical on every core; the mask data is
    all-zero on cores that don't own the slot.
    """
    gid = np.asarray(graph_ids, np.int64)
    slots = []           # (local_tile, graph)
    owners = []          # owning core
    for c in range(NCORES):
        g_c = gid[c * NLOC:(c + 1) * NLOC]
        for t in range(NT):
            lo = t * TILE
            hi = min((t + 1) * TILE, NLOC)
            if lo >= hi:
                continue
            gt = g_c[lo:hi]
            for g in np.unique(gt):
                slots.append((t, int(g)))
                owners.append(c)
    NSLOT = len(slots)
    gmaskb = np.zeros((NCORES, NSLOT, 128, 128), BF)
    for si, ((t, g), c) in enumerate(zip(slots, owners)):
        g_c = gid[c * NLOC:(c + 1) * NLOC]
        lo = t * TILE
        hi = min((t + 1) * TILE, NLOC)
        sel = (g_c[lo:hi] == g)
        m = np.zeros(128, np.float32)
        m[:hi - lo][sel] = 1.0
        gmaskb[c, si, :, :] = m[None, :].astype(BF)
    return slots, gmaskb


# ----------------------------------------------------------------------------- entry

_CACHE = {}

def _prepare(inputs, debug_taps=False):
    """Preprocess + build (cached on graph structure). Returns (nc, meta, in_maps)."""
    x = np.asarray(inputs["x"], np.float32)
    src = np.asarray(inputs["src"], np.int32)
    dst = np.asarray(inputs["dst"], np.int32)
    graph_ids = np.asarray(inputs["graph_ids"], np.int32)

    key = (hashlib.sha1(np.ascontiguousarray(src).tobytes()
                        + np.ascontiguousarray(dst).tobytes()
                        + np.ascontiguousarray(graph_ids).tobytes()).hexdigest(),
           debug_taps)
    if key in _CACHE:
        nc, meta = _CACHE[key]
        # x-dependent shard (xT) must be rebuilt if x changed; cheap enough to redo
        meta["shards"]["xT"] = _x_shard(x)
    else:
        meta, shards = _preprocess(x, src, dst, graph_ids)
        slots, gmaskb = _build_slots_and_masks(graph_ids)
        meta["slots"] = slots
        meta["gmaskb"] = gmaskb
        meta["shards"] = shards
        nc = build_program_v2(meta, debug_taps=debug_taps)
        _CACHE[key] = (nc, meta)
    shards = meta["shards"]
    gmaskb = meta["gmaskb"]

    w = _pack_weights(inputs["Win"], inputs["b_in"], inputs["Wsrc"], inputs["bsrc"],
                      inputs["Wdst"], inputs["bdst"], inputs["attn"],
                      inputs["Wc1"], inputs["bc1"], inputs["Wc2"], inputs["bc2"],
                      inputs["Wc3"], inputs["bc3"])

    in_maps = []
    for c in range(NCORES):
        m = dict(
            xT=shards["xT"][c], srcidx=shards["srcidx"][c], dstidx=shards["dstidx"][c],
            PT=shards["PT"][c], gmaskb=gmaskb[c],
            Win_bf=w["Win_bf"], Wsrc_pk=w["Wsrc_pk"], Wdst_pk=w["Wdst_pk"],
            attn_bc=w["attn_bc"], b_in_bc=w["b_in_bc"], bsrc_bc=w["bsrc_bc"],
            bdst_bc=w["bdst_bc"], Wc1_pk=w["Wc1_pk"], Wc2_pk=w["Wc2_pk"],
            Wc3_bf=w["Wc3_bf"], bc1_col=w["bc1_col"], bc2_col=w["bc2_col"],
            bc3_bc=w["bc3_bc"],
        )
        in_maps.append(m)
    return nc, meta, in_maps


def _x_shard(x):
    xT = np.zeros((NCORES, IN_DIM, NLOC_PAD), BF)
    for c in range(NCORES):
        xT[c, :, :NLOC] = x[c * NLOC:(c + 1) * NLOC].T.astype(BF)
    return xT


def kernel(**inputs):
    nc, meta, in_maps = _prepare(inputs)
    res = run_bass_kernel_spmd(nc, in_maps, core_ids=list(range(NCORES)))
    return np.asarray(res.results[0]["out"], np.float32)
